# revision 27
# baseline (speedup 1.0000x reference)
"""Distributed exact inner-product top-k (brute-force kNN) on 8 TRN2 NeuronCores.

Sharding: codebook W is split row-wise into 8 shards of 25000 (one per core);
x is replicated.  Host pre-transposes W to fp8e4 (halves the 6.4MB load
that otherwise starves the first batch group) and x to bf16 (the PE
streams 1 output column/cycle regardless of dtype at contraction 128 -
fp8 DoubleRow only pays off at contraction 256, measured - so the mixed
bf16 x is free accuracy).

Device kernel (SPMD, identical graph per core, no collectives):
  - per 1024-col region (2 PSUM banks, 4 in flight so both drain engines
    always have a ready region): 2x mixed bf16*fp8 matmuls
    [128 contraction, 512 cols] into PSUM (f32)
  - each region is drained by one of the only two engines that can read
    PSUM, statically balanced by their measured costs (DVE 1100ns/region
    at w=8 on 0.96GHz - the reduce has a per-output-window cost, so w=8
    beats w=4's 1145 - vs Act 1038ns at 1.2GHz; 1024/1024 regions are
    the only double-buffered split that fits the 8 PSUM banks):
      D: DVE windowed tensor_reduce(max) w=8 -> bf16 window maxima
         (odd regions except 23, plus 22: 12 regions)
      A: Act copy PSUM -> fp8e4 raw scores, window-1 (even regions, 23,
         and the cheap 424-col tail; Act owns region 0 so it starts first)
  - per-row outputs: 1536 bf16 w8-maxima + 12712 fp8 raw scores, DMA'd in
    multi-region chunks alternating between the SP and gpsimd DMA queues
    (~20MB/core total HBM traffic vs 37MB for an all-raw bf16 scheme)

Host merge (the all-gather + final top-k of the distributed ANN pattern):
  - per row, select every window whose (value + its route's EPS) clears
    (128th-largest window value - EPSMAX - slack); gather member columns
  - exact f64 re-rank of the candidates; final top-128 ordered like
    jax.lax.top_k (value desc, index asc)
  - exactness guard: containment holds if |device value - exact window max|
    <= EPS_route for every window that can matter; all such windows are
    selected, EPS is validated on them per-run, and violating rows
    (expected none) are recomputed exactly.  bf16 inputs keep the gap tiny
    (bf16-out windows ~0.25, fp8e4-out windows ~2.2 at |s|~45), so margins
    and candidate counts stay small and the host merge is cheap.
"""

import numpy as np

B = 1024
D = 128
VOCAB = 200000
NCORES = 8
VSHARD = VOCAB // NCORES  # 25000
REG = 1024  # 2 PSUM banks of f32
NREG = 24  # full regions per shard
TAIL = VSHARD - NREG * REG  # 424 -> Act
TOPK = 128
NGRP = B // 128

# Region schedule per group: DVE w8 reduce ("D") on odd regions (except
# 23) plus 22; Act fp8 copy ("A") on even regions, 23, and the tail.
# w=8 windows cost ~1100ns/region on DVE (vs 1145 at w=4 - the reduce has
# a per-output-window cost), which makes the 12D/12A+tail split the
# balance point: DVE ~13.2us vs Act ~13.0us per group (measured).
WIN = 16  # DVE window width
ROUTES = ["D" if (r % 2 == 1 and r != 23) or r == 22 else "A"
          for r in range(NREG)]
ROUTES.append("A")  # tail -> Act (cheap 526ns copy)
W4_SEGS = [(r * REG, REG) for r in range(NREG) if ROUTES[r] == "D"]
W1_SEGS = [(r * REG, REG) for r in range(NREG) if ROUTES[r] == "A"] + [
    (NREG * REG, TAIL)
]
NW4 = sum(n // WIN for _, n in W4_SEGS)  # 768
NW1 = sum(n for _, n in W1_SEGS)  # 12712
W4_OFF = np.concatenate([[0], np.cumsum([n // WIN for _, n in W4_SEGS])])
W1_OFF = np.concatenate([[0], np.cumsum([n for _, n in W1_SEGS])])

# |device window value - exact window max| bounds, validated at runtime:
# fp8e4 W quantization noise (x stays bf16) plus output quantization
# (bf16 ~0.2 for route D, fp8e4 ~2.2 for route A at |s|~45).
EPS4 = 2.2
EPS1 = 3.9
EPSMAX = EPS1
SLACK = 0.3

LAST_RESULTS = None  # BassKernelResults of the most recent run (for profiling)
_CACHED_NC = None


def build_kernel():
    import concourse.bass as bass  # noqa: F401
    import concourse.tile as tile
    from concourse import bacc, mybir

    F32 = mybir.dt.float32
    BF16 = mybir.dt.bfloat16
    FP8 = mybir.dt.float8e4
    AX = mybir.AxisListType.X
    MAX = mybir.AluOpType.max
    COPY = mybir.ActivationFunctionType.Copy

    nc = bacc.Bacc("TRN2", target_bir_lowering=False, debug=False)
    wt_d = nc.dram_tensor("wt", [D, VSHARD], FP8, kind="ExternalInput")
    xt_d = nc.dram_tensor("xt", [D, B], BF16, kind="ExternalInput")
    out4_d = nc.dram_tensor("out_w4", [B, NW4], BF16, kind="ExternalOutput")
    out1_d = nc.dram_tensor("out_w1", [B, NW1], FP8, kind="ExternalOutput")

    with tile.TileContext(nc) as tc:
        with (
            tc.tile_pool(name="wt", bufs=1) as wt_pool,
            tc.tile_pool(name="xt", bufs=1) as xt_pool,
            tc.tile_pool(name="psum", bufs=4, space="PSUM") as psum_pool,
            tc.tile_pool(name="out4", bufs=4) as out4_pool,
            tc.tile_pool(name="out1", bufs=4) as out1_pool,
        ):
            wt_sb = wt_pool.tile([D, VSHARD], FP8)
            xt_sb = xt_pool.tile([D, B], BF16)
            # xt on SP, first W slab concurrently on the gpsimd queue;
            # slabs arrive in consumption order so group 0 never waits.
            nc.sync.dma_start(xt_sb[:], xt_d[:])
            slabs = [512] * 8 + [1024] * 20 + [424]
            assert sum(slabs) == VSHARD
            lo = 0
            for s, w in enumerate(slabs):
                eng = nc.gpsimd if s % 2 == 0 else nc.sync
                eng.dma_start(wt_sb[:, lo:lo + w], wt_d[:, lo:lo + w])
                lo += w

            # Per-region output window offsets (segments are column-sorted
            # and regions are processed in column order, so offsets align).
            o4_of = {}
            o1_of = {}
            for si, (slo, n) in enumerate(W4_SEGS):
                o4_of[slo] = int(W4_OFF[si])
            for si, (slo, n) in enumerate(W1_SEGS):
                o1_of[slo] = int(W1_OFF[si])

            # DMA-out cuts: (after_region, stream, win_lo, win_hi)
            def done4(r):
                return sum(
                    n // WIN for slo, n in W4_SEGS if slo < (r + 1) * REG
                )

            def done1(r):
                return sum(n for slo, n in W1_SEGS if slo < (r + 1) * REG)

            cuts = []
            prev1 = 0
            for cr in (2, 6, 10, 14, 18, NREG):
                hi = done1(cr)
                cuts.append((cr, 1, prev1, hi))
                prev1 = hi
            prev4 = 0
            for cr in (13, 22):
                hi = done4(cr)
                cuts.append((cr, 4, prev4, hi))
                prev4 = hi

            for g in range(NGRP):
                out4_sb = out4_pool.tile([128, NW4], BF16, tag="out4")
                out1_sb = out1_pool.tile([128, NW1], FP8, tag="out1")
                xg = xt_sb[:, g * 128:(g + 1) * 128]
                for r in range(NREG + 1):
                    base = r * REG
                    w_cols = REG if r < NREG else TAIL
                    route = ROUTES[r]
                    ps = psum_pool.tile([128, REG], F32)
                    for k in range(0, w_cols, 512):
                        kw = min(512, w_cols - k)
                        nc.tensor.matmul(
                            ps[:, k:k + kw],
                            xg,
                            wt_sb[:, base + k:base + k + kw],
                            start=True, stop=True,
                        )
                    if route == "D":
                        o4 = o4_of[base]
                        nc.vector.tensor_reduce(
                            out4_sb[:, o4:o4 + w_cols // WIN],
                            ps[:, :w_cols].rearrange(
                                "p (n w) -> p n w", w=WIN),
                            axis=AX, op=MAX,
                        )
                    else:
                        o1 = o1_of[base]
                        nc.scalar.activation(
                            out1_sb[:, o1:o1 + w_cols],
                            ps[:, :w_cols],
                            COPY,
                        )
                    for ci, (cr, which, wlo, whi) in enumerate(cuts):
                        if cr != r:
                            continue
                        eng = nc.sync if (g + ci) % 2 == 0 else nc.gpsimd
                        src_t = out4_sb if which == 4 else out1_sb
                        dst = out4_d if which == 4 else out1_d
                        eng.dma_start(
                            dst[g * 128:(g + 1) * 128, wlo:whi],
                            src_t[:, wlo:whi],
                        )
    nc.compile()
    return nc


def _build_maps():
    """Per-window candidate columns and EPS.

    Returns (colmap [NWIN, 4] int64 with -1 pads, eps [NWIN] f32) where
    window order is [all w4 windows, all w1 windows] per core.
    """
    nwin = NW4 + NW1
    cm = np.full((nwin, WIN), -1, np.int64)
    eps = np.empty(nwin, np.float32)
    for si, (lo, n) in enumerate(W4_SEGS):
        o = int(W4_OFF[si])
        j = np.arange(n // WIN)[:, None]
        cm[o:o + n // WIN] = lo + WIN * j + np.arange(WIN)[None, :]
    eps[:NW4] = EPS4
    for si, (lo, n) in enumerate(W1_SEGS):
        o = NW4 + int(W1_OFF[si])
        cm[o:o + n, 0] = lo + np.arange(n)
    eps[NW4:] = EPS1
    return cm, eps


_COLMAP, _WEPS = _build_maps()


def _topk_rows(vals, gidx, k):
    """Per-row top-k ordered like jax.lax.top_k: value desc, index asc."""
    order = np.lexsort((gidx, -vals), axis=-1)[:, :k]
    return (
        np.take_along_axis(gidx, order, axis=1),
        np.take_along_axis(vals, order, axis=1),
    )


def kernel(x: np.ndarray, W: np.ndarray, topk) -> np.ndarray:
    global LAST_RESULTS, _CACHED_NC
    import os

    import ml_dtypes

    from concourse.bass_utils import run_bass_kernel_spmd

    assert x.shape == (B, D) and W.shape == (VOCAB, D)
    assert int(topk) == TOPK
    x = np.ascontiguousarray(np.asarray(x, dtype=np.float32))
    W = np.ascontiguousarray(np.asarray(W, dtype=np.float32))

    if _CACHED_NC is None:
        _CACHED_NC = build_kernel()
    nc = _CACHED_NC

    xt = np.ascontiguousarray(x.T).astype(ml_dtypes.bfloat16)
    in_maps = []
    for i in range(NCORES):
        wt_i = np.ascontiguousarray(
            W[i * VSHARD:(i + 1) * VSHARD].T
        ).astype(ml_dtypes.float8_e4m3)
        in_maps.append({"wt": wt_i, "xt": xt})

    LAST_RESULTS = run_bass_kernel_spmd(
        nc,
        in_maps,
        core_ids=list(range(NCORES)),
        trace=bool(int(os.environ.get("KERNEL_TRACE", "0"))),
    )
    results = LAST_RESULTS.results

    # [B, 8*(NW4+NW1)] device window values, f32
    nwin = NW4 + NW1
    wm = np.empty((B, NCORES * nwin), np.float32)
    for i in range(NCORES):
        wm[:, i * nwin:i * nwin + NW4] = np.asarray(
            results[i]["out_w4"]).astype(np.float32)
        wm[:, i * nwin + NW4:(i + 1) * nwin] = np.asarray(
            results[i]["out_w1"]).astype(np.float32)
    nwin_all = NCORES * nwin
    weps_all = np.tile(_WEPS, NCORES)

    # Per-row selection on adjusted values v' = v + eps_w:
    # keep windows with v' >= kth_dev - EPSMAX - SLACK.
    wma = wm + weps_all[None, :]
    kth = np.partition(wm, nwin_all - TOPK, axis=1)[:, nwin_all - TOPK]
    tau = kth - EPSMAX - SLACK
    counts = (wma >= tau[:, None]).sum(axis=1)
    K = int(min(max(int(counts.max()), TOPK + 64), 3072))
    topw = np.argpartition(-wma, K - 1, axis=1)[:, :K]  # [B, K] window ids

    core_id = topw // nwin
    wi = topw % nwin
    cols = _COLMAP[wi]  # [B, K, WIN], -1 = pad
    pad = cols < 0
    cand = (np.where(pad, 0, cols)
            + core_id[..., None] * VSHARD).reshape(B, K * WIN)

    # Exact f64 re-rank of the candidate columns (pads scored -inf).
    x64 = x.astype(np.float64)
    W64 = W.astype(np.float64)
    exact = np.empty((B, K * WIN), np.float64)
    STEP = 64
    for r0 in range(0, B, STEP):
        r1 = r0 + STEP
        gW = W64[cand[r0:r1]]  # [STEP, K*WIN, D]
        exact[r0:r1] = np.einsum("bjd,bd->bj", gW, x64[r0:r1])
    exact[pad.reshape(B, K * WIN)] = -np.inf

    # Rank on f32-rounded scores so near-ties break the same way as the
    # f32 reference (top_k on an f32 score matrix, ties by index asc).
    gidx_top, vals_top = _topk_rows(
        exact.astype(np.float32).astype(np.float64), cand, TOPK)

    # Exactness guards: EPS must hold on every selected window (any window
    # that can contain a true top-128 column is selected), and the
    # selection count must fit in K.
    dev_w = np.take_along_axis(wm, topw, axis=1)
    true_w = exact.reshape(B, K, WIN).max(axis=2)
    werr = np.abs(dev_w - true_w)
    sel_eps = weps_all[topw]
    err_excess = (werr - sel_eps).max(axis=1)
    bad = (err_excess > 0) | (counts > K)
    if os.environ.get("KERNEL_DEBUG"):
        w4mask = (topw % nwin) < NW4
        e4 = werr[w4mask].max() if w4mask.any() else 0.0
        e1 = werr[~w4mask].max() if (~w4mask).any() else 0.0
        print(f"[kernel] K={K} counts max={counts.max()} "
              f"err4 max={e4:.3f} err1 max={e1:.3f} bad rows={int(bad.sum())}")
    for r in np.flatnonzero(bad):
        s = x64[r] @ W64.T
        gidx_top[r] = np.lexsort((np.arange(VOCAB), -s))[:TOPK]

    return gidx_top.astype(np.int32)


# revision 28
# speedup vs baseline: 1.1893x; 1.1893x over previous
"""Distributed exact inner-product top-k (brute-force kNN) on 8 TRN2 NeuronCores.

Sharding: codebook W is split row-wise into 8 shards of 25000 (one per core);
x is replicated.  Host pre-transposes W to fp8e4 (halves the 6.4MB load
that otherwise starves the first batch group) and x to bf16 (the PE
streams 1 output column/cycle regardless of dtype at contraction 128 -
fp8 DoubleRow only pays off at contraction 256, measured - so the mixed
bf16 x is free accuracy).

Device kernel (SPMD, identical graph per core, no collectives):
  - per 1024-col region (2 PSUM banks, 4 in flight so both drain engines
    always have a ready region): 2x mixed bf16*fp8 matmuls
    [128 contraction, 512 cols] into PSUM (f32)
  - each region is drained by one of the only two engines that can read
    PSUM, statically balanced by their measured costs (DVE 1100ns/region
    at w=8 on 0.96GHz - the reduce has a per-output-window cost, so w=8
    beats w=4's 1145 - vs Act 1038ns at 1.2GHz; 1024/1024 regions are
    the only double-buffered split that fits the 8 PSUM banks):
      D: DVE windowed tensor_reduce(max) w=8 -> bf16 window maxima
         (odd regions except 23, plus 22: 12 regions)
      A: Act copy PSUM -> fp8e4 raw scores, window-1 (even regions, 23,
         and the cheap 424-col tail; Act owns region 0 so it starts first)
  - per-row outputs: 1536 bf16 w8-maxima + 12712 fp8 raw scores, DMA'd in
    multi-region chunks alternating between the SP and gpsimd DMA queues
    (~20MB/core total HBM traffic vs 37MB for an all-raw bf16 scheme)

Host merge (the all-gather + final top-k of the distributed ANN pattern):
  - per row, select every window whose (value + its route's EPS) clears
    (128th-largest window value - EPSMAX - slack); gather member columns
  - exact f64 re-rank of the candidates; final top-128 ordered like
    jax.lax.top_k (value desc, index asc)
  - exactness guard: containment holds if |device value - exact window max|
    <= EPS_route for every window that can matter; all such windows are
    selected, EPS is validated on them per-run, and violating rows
    (expected none) are recomputed exactly.  bf16 inputs keep the gap tiny
    (bf16-out windows ~0.25, fp8e4-out windows ~2.2 at |s|~45), so margins
    and candidate counts stay small and the host merge is cheap.
"""

import numpy as np

B = 1024
D = 128
VOCAB = 200000
NCORES = 8
VSHARD = VOCAB // NCORES  # 25000
REG = 1024  # 2 PSUM banks of f32
NREG = 24  # full regions per shard
TAIL = VSHARD - NREG * REG  # 424 -> Act
TOPK = 128
NGRP = B // 128

# Region schedule per group: DVE w8 reduce ("D") on odd regions (except
# 23) plus 22; Act fp8 copy ("A") on even regions, 23, and the tail.
# w=8 windows cost ~1100ns/region on DVE (vs 1145 at w=4 - the reduce has
# a per-output-window cost), which makes the 12D/12A+tail split the
# balance point: DVE ~13.2us vs Act ~13.0us per group (measured).
WIN = 8  # DVE window width
ROUTES = ["D" if (r % 2 == 1 and r != 23) or r == 22 else "A"
          for r in range(NREG)]
ROUTES.append("A")  # tail -> Act (cheap 526ns copy)
W4_SEGS = [(r * REG, REG) for r in range(NREG) if ROUTES[r] == "D"]
W1_SEGS = [(r * REG, REG) for r in range(NREG) if ROUTES[r] == "A"] + [
    (NREG * REG, TAIL)
]
NW4 = sum(n // WIN for _, n in W4_SEGS)  # 1536
NW1 = sum(n for _, n in W1_SEGS)  # 12712
W4_OFF = np.concatenate([[0], np.cumsum([n // WIN for _, n in W4_SEGS])])
W1_OFF = np.concatenate([[0], np.cumsum([n for _, n in W1_SEGS])])

# |device window value - exact window max| bounds, validated at runtime:
# fp8e4 W quantization noise (x stays bf16) plus output quantization
# (bf16 ~0.2 for route D, fp8e4 ~2.2 for route A at |s|~45).
EPS4 = 2.2
EPS1 = 3.9
EPSMAX = EPS1
SLACK = 0.3

LAST_RESULTS = None  # BassKernelResults of the most recent run (for profiling)
_CACHED_NC = None


def build_kernel():
    import concourse.bass as bass  # noqa: F401
    import concourse.tile as tile
    from concourse import bacc, mybir

    F32 = mybir.dt.float32
    BF16 = mybir.dt.bfloat16
    FP8 = mybir.dt.float8e4
    AX = mybir.AxisListType.X
    MAX = mybir.AluOpType.max
    COPY = mybir.ActivationFunctionType.Copy

    nc = bacc.Bacc("TRN2", target_bir_lowering=False, debug=False)
    wt_d = nc.dram_tensor("wt", [D, VSHARD], FP8, kind="ExternalInput")
    xt_d = nc.dram_tensor("xt", [D, B], BF16, kind="ExternalInput")
    out4_d = nc.dram_tensor("out_w4", [B, NW4], BF16, kind="ExternalOutput")
    out1_d = nc.dram_tensor("out_w1", [B, NW1], FP8, kind="ExternalOutput")

    with tile.TileContext(nc) as tc:
        with (
            tc.tile_pool(name="wt", bufs=1) as wt_pool,
            tc.tile_pool(name="xt", bufs=1) as xt_pool,
            tc.tile_pool(name="psum", bufs=4, space="PSUM") as psum_pool,
            tc.tile_pool(name="out4", bufs=4) as out4_pool,
            tc.tile_pool(name="out1", bufs=4) as out1_pool,
        ):
            wt_sb = wt_pool.tile([D, VSHARD], FP8)
            xt_sb = xt_pool.tile([D, B], BF16)
            # xt on SP, first W slab concurrently on the gpsimd queue;
            # slabs arrive in consumption order so group 0 never waits.
            nc.sync.dma_start(xt_sb[:], xt_d[:])
            slabs = [512] * 8 + [1024] * 20 + [424]
            assert sum(slabs) == VSHARD
            lo = 0
            for s, w in enumerate(slabs):
                eng = nc.gpsimd if s % 2 == 0 else nc.sync
                eng.dma_start(wt_sb[:, lo:lo + w], wt_d[:, lo:lo + w])
                lo += w

            # Per-region output window offsets (segments are column-sorted
            # and regions are processed in column order, so offsets align).
            o4_of = {}
            o1_of = {}
            for si, (slo, n) in enumerate(W4_SEGS):
                o4_of[slo] = int(W4_OFF[si])
            for si, (slo, n) in enumerate(W1_SEGS):
                o1_of[slo] = int(W1_OFF[si])

            # DMA-out cuts: (after_region, stream, win_lo, win_hi)
            def done4(r):
                return sum(
                    n // WIN for slo, n in W4_SEGS if slo < (r + 1) * REG
                )

            def done1(r):
                return sum(n for slo, n in W1_SEGS if slo < (r + 1) * REG)

            cuts = []
            prev1 = 0
            for cr in (2, 6, 10, 14, 18, NREG):
                hi = done1(cr)
                cuts.append((cr, 1, prev1, hi))
                prev1 = hi
            prev4 = 0
            for cr in (13, 22):
                hi = done4(cr)
                cuts.append((cr, 4, prev4, hi))
                prev4 = hi

            for g in range(NGRP):
                out4_sb = out4_pool.tile([128, NW4], BF16, tag="out4")
                out1_sb = out1_pool.tile([128, NW1], FP8, tag="out1")
                xg = xt_sb[:, g * 128:(g + 1) * 128]
                for r in range(NREG + 1):
                    base = r * REG
                    w_cols = REG if r < NREG else TAIL
                    route = ROUTES[r]
                    ps = psum_pool.tile([128, REG], F32)
                    for k in range(0, w_cols, 512):
                        kw = min(512, w_cols - k)
                        nc.tensor.matmul(
                            ps[:, k:k + kw],
                            xg,
                            wt_sb[:, base + k:base + k + kw],
                            start=True, stop=True,
                        )
                    if route == "D":
                        o4 = o4_of[base]
                        nc.vector.tensor_reduce(
                            out4_sb[:, o4:o4 + w_cols // WIN],
                            ps[:, :w_cols].rearrange(
                                "p (n w) -> p n w", w=WIN),
                            axis=AX, op=MAX,
                        )
                    else:
                        o1 = o1_of[base]
                        nc.scalar.activation(
                            out1_sb[:, o1:o1 + w_cols],
                            ps[:, :w_cols],
                            COPY,
                        )
                    for ci, (cr, which, wlo, whi) in enumerate(cuts):
                        if cr != r:
                            continue
                        eng = nc.sync if (g + ci) % 2 == 0 else nc.gpsimd
                        src_t = out4_sb if which == 4 else out1_sb
                        dst = out4_d if which == 4 else out1_d
                        eng.dma_start(
                            dst[g * 128:(g + 1) * 128, wlo:whi],
                            src_t[:, wlo:whi],
                        )
    nc.compile()
    return nc


def _build_maps():
    """Per-window candidate columns and EPS.

    Returns (colmap [NWIN, 4] int64 with -1 pads, eps [NWIN] f32) where
    window order is [all w4 windows, all w1 windows] per core.
    """
    nwin = NW4 + NW1
    cm = np.full((nwin, WIN), -1, np.int64)
    eps = np.empty(nwin, np.float32)
    for si, (lo, n) in enumerate(W4_SEGS):
        o = int(W4_OFF[si])
        j = np.arange(n // WIN)[:, None]
        cm[o:o + n // WIN] = lo + WIN * j + np.arange(WIN)[None, :]
    eps[:NW4] = EPS4
    for si, (lo, n) in enumerate(W1_SEGS):
        o = NW4 + int(W1_OFF[si])
        cm[o:o + n, 0] = lo + np.arange(n)
    eps[NW4:] = EPS1
    return cm, eps


_COLMAP, _WEPS = _build_maps()


def _topk_rows(vals, gidx, k):
    """Per-row top-k ordered like jax.lax.top_k: value desc, index asc."""
    order = np.lexsort((gidx, -vals), axis=-1)[:, :k]
    return (
        np.take_along_axis(gidx, order, axis=1),
        np.take_along_axis(vals, order, axis=1),
    )


def kernel(x: np.ndarray, W: np.ndarray, topk) -> np.ndarray:
    global LAST_RESULTS, _CACHED_NC
    import os

    import ml_dtypes

    from concourse.bass_utils import run_bass_kernel_spmd

    assert x.shape == (B, D) and W.shape == (VOCAB, D)
    assert int(topk) == TOPK
    x = np.ascontiguousarray(np.asarray(x, dtype=np.float32))
    W = np.ascontiguousarray(np.asarray(W, dtype=np.float32))

    if _CACHED_NC is None:
        _CACHED_NC = build_kernel()
    nc = _CACHED_NC

    xt = np.ascontiguousarray(x.T).astype(ml_dtypes.bfloat16)
    in_maps = []
    for i in range(NCORES):
        wt_i = np.ascontiguousarray(
            W[i * VSHARD:(i + 1) * VSHARD].T
        ).astype(ml_dtypes.float8_e4m3)
        in_maps.append({"wt": wt_i, "xt": xt})

    LAST_RESULTS = run_bass_kernel_spmd(
        nc,
        in_maps,
        core_ids=list(range(NCORES)),
        trace=bool(int(os.environ.get("KERNEL_TRACE", "0"))),
    )
    results = LAST_RESULTS.results

    # [B, 8*(NW4+NW1)] device window values, f32
    nwin = NW4 + NW1
    wm = np.empty((B, NCORES * nwin), np.float32)
    for i in range(NCORES):
        wm[:, i * nwin:i * nwin + NW4] = np.asarray(
            results[i]["out_w4"]).astype(np.float32)
        wm[:, i * nwin + NW4:(i + 1) * nwin] = np.asarray(
            results[i]["out_w1"]).astype(np.float32)
    nwin_all = NCORES * nwin
    weps_all = np.tile(_WEPS, NCORES)

    # Per-row selection on adjusted values v' = v + eps_w:
    # keep windows with v' >= kth_dev - EPSMAX - SLACK.
    wma = wm + weps_all[None, :]
    kth = np.partition(wm, nwin_all - TOPK, axis=1)[:, nwin_all - TOPK]
    tau = kth - EPSMAX - SLACK
    counts = (wma >= tau[:, None]).sum(axis=1)
    K = int(min(max(int(counts.max()), TOPK + 64), 4096))
    topw = np.argpartition(-wma, K - 1, axis=1)[:, :K]  # [B, K] window ids

    core_id = topw // nwin
    wi = topw % nwin
    cols = _COLMAP[wi]  # [B, K, WIN], -1 = pad
    pad = cols < 0
    cand = (np.where(pad, 0, cols)
            + core_id[..., None] * VSHARD).reshape(B, K * WIN)

    # Exact f64 re-rank of the candidate columns (pads scored -inf).
    x64 = x.astype(np.float64)
    W64 = W.astype(np.float64)
    exact = np.empty((B, K * WIN), np.float64)
    STEP = 64
    for r0 in range(0, B, STEP):
        r1 = r0 + STEP
        gW = W64[cand[r0:r1]]  # [STEP, K*WIN, D]
        exact[r0:r1] = np.einsum("bjd,bd->bj", gW, x64[r0:r1])
    exact[pad.reshape(B, K * WIN)] = -np.inf

    # Rank on f32-rounded scores so near-ties break the same way as the
    # f32 reference (top_k on an f32 score matrix, ties by index asc).
    gidx_top, vals_top = _topk_rows(
        exact.astype(np.float32).astype(np.float64), cand, TOPK)

    # Exactness guards: EPS must hold on every selected window (any window
    # that can contain a true top-128 column is selected), and the
    # selection count must fit in K.
    dev_w = np.take_along_axis(wm, topw, axis=1)
    true_w = exact.reshape(B, K, WIN).max(axis=2)
    werr = np.abs(dev_w - true_w)
    sel_eps = weps_all[topw]
    err_excess = (werr - sel_eps).max(axis=1)
    bad = (err_excess > 0) | (counts > K)
    if os.environ.get("KERNEL_DEBUG"):
        w4mask = (topw % nwin) < NW4
        e4 = werr[w4mask].max() if w4mask.any() else 0.0
        e1 = werr[~w4mask].max() if (~w4mask).any() else 0.0
        print(f"[kernel] K={K} counts max={counts.max()} "
              f"err4 max={e4:.3f} err1 max={e1:.3f} bad rows={int(bad.sum())}")
    for r in np.flatnonzero(bad):
        s = x64[r] @ W64.T
        gidx_top[r] = np.lexsort((np.arange(VOCAB), -s))[:TOPK]

    return gidx_top.astype(np.int32)
